# revision 5
# baseline (speedup 1.0000x reference)
"""Multi-head attention (B=2,S=2048,E=1024,H=16,DK=DV=64) on 8 Trainium2 cores.

Sharding: core c handles batch c//4 and head-group c%4 (4 heads each).
Each core computes q/k/v projections for its heads, masked softmax attention
(transposed-scores layout), and a partial output projection with its rows of
Wo.  The host sums the 4 partial outputs per batch and adds bo.

All matmuls run in bf16 with fp32 PSUM accumulation.  Softmax skips the
running-max (scores are O(1) here; a constant -3 bias in the exp guards the
range), masking is a bf16 multiply after exp, and the softmax denominator is
produced by a ones-column appended to V inside the ctx matmul.
"""

import numpy as np
import ml_dtypes

import concourse.bacc as bacc
import concourse.mybir as mybir
import concourse.tile as tile
from concourse import bass_utils

BF = ml_dtypes.bfloat16
dt = mybir.dt

NCORES = 8


def _emit(nc, tc, inp, y_d, S, E, HL, DK):
    EC = E // 128          # contraction chunks for the projections
    NT = S // 128          # seq tiles
    DKL = HL * DK          # local head dims (256)
    NP = DKL // 128        # q/k partition tiles (pairs of heads)
    Exp = mybir.ActivationFunctionType.Exp
    Copy = mybir.ActivationFunctionType.Copy
    CS = min(512, S)   # matmul free-dim chunk over seq
    CE = min(512, E)   # matmul free-dim chunk over embed

    persist = tc.alloc_tile_pool(name="persist", bufs=1)
    qT = [persist.tile([128, S], dt.bfloat16, name=f"qT{m}") for m in range(NP)]
    kT = [persist.tile([128, S], dt.bfloat16, name=f"kT{m}") for m in range(NP)]
    cT = [persist.tile([128, S], dt.bfloat16, name=f"cT{m}") for m in range(NP)]
    vA = [persist.tile([128, HL * (DK + 1)], dt.bfloat16, name=f"vA{t}")
          for t in range(NT)]
    ones = persist.tile([1, S], dt.bfloat16, name="ones")
    nc.gpsimd.memset(ones[:], 1.0)
    neg3 = persist.tile([128, 1], dt.float32, name="neg3")
    nc.gpsimd.memset(neg3[:], -3.0)

    w_sb = {}
    b_sb = {}
    for nm in ("wq", "wk", "wv"):
        w_sb[nm] = [persist.tile([128, DKL], dt.bfloat16, name=f"{nm}{c}")
                    for c in range(EC)]
        for c in range(EC):
            nc.sync.dma_start(w_sb[nm][c][:], inp[nm][c])
    for nm in ("bq", "bk", "bv"):
        b_sb[nm] = persist.tile([1, DKL], dt.bfloat16, name=f"{nm}s")
        nc.sync.dma_start(b_sb[nm][:], inp[nm][:])
    wo_sb = [persist.tile([128, E], dt.bfloat16, name=f"wo{p}") for p in range(NP)]
    for p in range(NP):
        nc.sync.dma_start(wo_sb[p][:], inp["wo"][p])

    # ---- phase 1a: q^T and k^T projections ----------------------------------
    with tc.tile_pool(name="xs", bufs=12) as xs_pool:
        with tc.tile_pool(name="qkps", bufs=2, space="PSUM") as qkps:
            for nm, bias_nm, scale, outtiles in (
                ("xq", "bq", 1.0 / np.sqrt(DK), qT),
                ("xk", "bk", 1.0, kT),
            ):
                xt = []
                for c in range(EC):
                    t_ = xs_pool.tile([128, S], dt.bfloat16, tag="xs",
                                      name=f"{nm}{c}")
                    nc.sync.dma_start(t_[:], inp[nm][c])
                    xt.append(t_)
                for m in range(NP):
                    ps = qkps.tile([128, S], dt.float32, tag="qkps",
                                   name=f"{nm}ps{m}")
                    for n0 in range(0, S, CS):
                        nc.tensor.matmul(
                            ps[:, n0:n0 + CS],
                            b_sb[bias_nm][0:1, 128 * m:128 * (m + 1)],
                            ones[0:1, n0:n0 + CS],
                            start=True, stop=False)
                    for c in range(EC):
                        for n0 in range(0, S, CS):
                            nc.tensor.matmul(
                                ps[:, n0:n0 + CS],
                                w_sb[nm.replace("x", "w")][c][:, 128 * m:128 * (m + 1)],
                                xt[c][:, n0:n0 + CS],
                                start=False, stop=(c == EC - 1))
                    nc.scalar.activation(outtiles[m][:], ps[:], Copy, scale=scale)

        # ---- phase 1b: v (natural layout) with ones column ------------------
        with tc.tile_pool(name="vps", bufs=2, space="PSUM") as vpsp:
            xt = []
            for c in range(EC):
                t_ = xs_pool.tile([128, S], dt.bfloat16, tag="xs", name=f"xv{c}")
                nc.sync.dma_start(t_[:], inp["xv"][c])
                xt.append(t_)
            for t in range(NT):
                vps = vpsp.tile([128, DKL], dt.float32, tag="vps", name=f"vps{t}")
                nc.tensor.matmul(vps[:], ones[0:1, 0:128], b_sb["bv"][:],
                                 start=True, stop=False)
                for c in range(EC):
                    nc.tensor.matmul(vps[:],
                                     xt[c][:, t * 128:(t + 1) * 128],
                                     w_sb["wv"][c][:],
                                     start=False, stop=(c == EC - 1))
                nc.gpsimd.memset(vA[t][:], 1.0)
                nc.vector.tensor_copy(
                    vA[t][:].rearrange("p (h c) -> p h c", h=HL)[:, :, 0:DK],
                    vps[:].rearrange("p (h c) -> p h c", h=HL))

    # ---- phase 2: attention per head ---------------------------------------
    with tc.tile_pool(name="mask", bufs=NT) as mpool, \
         tc.tile_pool(name="est", bufs=3) as epool, \
         tc.tile_pool(name="nrm", bufs=2) as npool, \
         tc.tile_pool(name="stps", bufs=2, space="PSUM") as stpool, \
         tc.tile_pool(name="ctxps", bufs=1, space="PSUM") as ctxpool:
        mt = []
        for t in range(NT):
            t_ = mpool.tile([128, S], dt.bfloat16, tag="mask", name=f"mask{t}")
            nc.sync.dma_start(t_[:], inp["mask"][:, t, :])
            mt.append(t_)

        HB = min(1024, S)  # qi half-block so st psum is 2 banks
        for h in range(HL):
            pair, sub = h // 2, (h % 2) * 64
            ctx = ctxpool.tile([DK + 1, S], dt.float32, tag="ctx", name=f"ctx{h}")
            for t in range(NT):
                for q0 in range(0, S, HB):
                    st = stpool.tile([128, HB], dt.float32, tag="st",
                                     name=f"st{h}_{t}_{q0}")
                    for n0 in range(0, HB, CS):
                        nc.tensor.matmul(
                            st[:, n0:n0 + CS],
                            kT[pair][sub:sub + DK, t * 128:(t + 1) * 128],
                            qT[pair][sub:sub + DK, q0 + n0:q0 + n0 + CS],
                            start=True, stop=True)
                    e = epool.tile([128, HB], dt.bfloat16, tag="e",
                                   name=f"e{h}_{t}_{q0}")
                    nc.scalar.activation(e[:], st[:], Exp, bias=neg3[:])
                    nc.vector.tensor_mul(e[:], e[:], mt[t][:, q0:q0 + HB])
                    for n0 in range(0, HB, CS):
                        nc.tensor.matmul(
                            ctx[:, q0 + n0:q0 + n0 + CS],
                            vA[t][:, h * (DK + 1):(h + 1) * (DK + 1)],
                            e[:, n0:n0 + CS],
                            start=(t == 0), stop=(t == NT - 1))
            r = npool.tile([1, S], dt.float32, tag="r", name=f"r{h}")
            nc.vector.reciprocal(r[:], ctx[DK:DK + 1, :])
            bc = npool.tile([DK, S], dt.float32, tag="bc", name=f"bc{h}")
            nc.gpsimd.partition_broadcast(bc[:], r[:])
            nc.vector.tensor_mul(cT[pair][sub:sub + DK, :], ctx[0:DK, :], bc[:])

    # ---- phase 3: partial output projection --------------------------------
    with tc.tile_pool(name="yps", bufs=2, space="PSUM") as ypool, \
         tc.tile_pool(name="ysb", bufs=2) as ysbp:
        for s in range(NT):
            yps = ypool.tile([128, E], dt.float32, tag="y", name=f"yps{s}")
            for p in range(NP):
                for n0 in range(0, E, CE):
                    nc.tensor.matmul(yps[:, n0:n0 + CE],
                                     cT[p][:, s * 128:(s + 1) * 128],
                                     wo_sb[p][:, n0:n0 + CE],
                                     start=(p == 0), stop=(p == NP - 1))
            ysb = ysbp.tile([128, E], dt.float32, tag="ysb", name=f"ysb{s}")
            if s % 2 == 0:
                nc.scalar.activation(ysb[:], yps[:], Copy)
            else:
                nc.vector.tensor_copy(ysb[:], yps[:])
            nc.sync.dma_start(y_d[s * 128:(s + 1) * 128, :], ysb[:])

    persist.release()


def _build(S, E, HL, DK):
    EC = E // 128
    NT = S // 128
    DKL = HL * DK
    NP = DKL // 128
    nc = bacc.Bacc("TRN2", target_bir_lowering=False, debug=False,
                   num_devices=NCORES)
    inp = {}
    for nm in ("xq", "xk", "xv"):
        inp[nm] = nc.dram_tensor(nm, [EC, 128, S], dt.bfloat16,
                                 kind="ExternalInput").ap()
    for nm in ("wq", "wk", "wv"):
        inp[nm] = nc.dram_tensor(nm, [EC, 128, DKL], dt.bfloat16,
                                 kind="ExternalInput").ap()
    for nm in ("bq", "bk", "bv"):
        inp[nm] = nc.dram_tensor(nm, [1, DKL], dt.bfloat16,
                                 kind="ExternalInput").ap()
    inp["wo"] = nc.dram_tensor("wo", [NP, 128, E], dt.bfloat16,
                               kind="ExternalInput").ap()
    inp["mask"] = nc.dram_tensor("mask", [128, NT, S], dt.bfloat16,
                                 kind="ExternalInput").ap()
    y_d = nc.dram_tensor("y", [S, E], dt.float32, kind="ExternalOutput").ap()

    with tile.TileContext(nc) as tc:
        _emit(nc, tc, inp, y_d, S, E, HL, DK)
    nc.compile()
    return nc


_CACHE = {}
_TRACE = False
_TRACE_CORES = (0,)
_LAST_RESULT = None


def _get_nc(S, E, HL, DK):
    key = (S, E, HL, DK)
    if key not in _CACHE:
        _CACHE[key] = _build(S, E, HL, DK)
    return _CACHE[key]


def run_sharded(query, key, value, mask, Wq, bq, Wk, bk, Wv, bv, Wo, bo):
    """Full-input -> full-output runner (generic shapes)."""
    global _LAST_RESULT
    query, key, value = (np.asarray(a, np.float32) for a in (query, key, value))
    mask = np.asarray(mask)
    Wq, bq, Wk, bk, Wv, bv, Wo, bo = (
        np.asarray(a, np.float32) for a in (Wq, bq, Wk, bk, Wv, bv, Wo, bo))

    B, S, E = query.shape
    HDK = Wq.shape[1]
    DKv = 64
    H = HDK // DKv
    GPB = NCORES // B                 # cores per batch
    HL = H // GPB                     # heads per core
    DKL = HL * DKv
    EC = E // 128
    NT = S // 128

    nc = _get_nc(S, E, HL, DKv)

    # per-batch host prep (shared by the 4 cores of a batch)
    xb = {}
    for b in range(B):
        xb[b] = {
            "xq": np.ascontiguousarray(query[b].T).astype(BF).reshape(EC, 128, S),
            "xk": np.ascontiguousarray(key[b].T).astype(BF).reshape(EC, 128, S),
            "xv": np.ascontiguousarray(value[b].T).astype(BF).reshape(EC, 128, S),
            "mask": np.ascontiguousarray(
                mask[b].reshape(S, NT, 128).transpose(2, 1, 0)).astype(BF),
        }

    in_maps = []
    for c in range(NCORES):
        b, g = c // GPB, c % GPB
        sl = slice(g * DKL, (g + 1) * DKL)
        in_maps.append({
            **xb[b],
            "wq": np.ascontiguousarray(Wq[:, sl]).astype(BF).reshape(EC, 128, DKL),
            "wk": np.ascontiguousarray(Wk[:, sl]).astype(BF).reshape(EC, 128, DKL),
            "wv": np.ascontiguousarray(Wv[:, sl]).astype(BF).reshape(EC, 128, DKL),
            "bq": bq[sl].astype(BF).reshape(1, DKL),
            "bk": bk[sl].astype(BF).reshape(1, DKL),
            "bv": bv[sl].astype(BF).reshape(1, DKL),
            "wo": np.ascontiguousarray(Wo[sl, :]).astype(BF).reshape(
                DKL // 128, 128, E),
        })

    kwargs = {}
    if _TRACE:
        kwargs = dict(trace=True, trace_cores=list(_TRACE_CORES))
    res = bass_utils.run_bass_kernel_spmd(nc, in_maps,
                                          core_ids=list(range(NCORES)), **kwargs)
    _LAST_RESULT = res

    y = np.zeros((B, S, E), np.float32)
    for c in range(NCORES):
        y[c // GPB] += res.results[c]["y"]
    y += bo.astype(np.float32)
    return y


def kernel(**inputs):
    return run_sharded(
        inputs["query"], inputs["key"], inputs["value"], inputs["mask"],
        inputs["Wq"], inputs["bq"], inputs["Wk"], inputs["bk"],
        inputs["Wv"], inputs["bv"], inputs["Wo"], inputs["bo"])


# revision 11
# speedup vs baseline: 1.1866x; 1.1866x over previous
"""Multi-head attention (B=2,S=2048,E=1024,H=16,DK=DV=64) on 8 Trainium2 cores.

Sharding: core c handles batch c//4 and head-group c%4 (4 heads each).
Each core computes q/k/v projections for its heads, masked softmax attention
(transposed-scores layout), and a partial output projection with its rows of
Wo.  The host sums the 4 partial outputs per batch and adds bo.

All matmuls run in bf16 with fp32 PSUM accumulation.  Softmax skips the
running-max (scores are O(1) here; a constant -3 bias in the exp guards the
range), masking is a bf16 multiply after exp, and the softmax denominator is
produced by a ones-column appended to V inside the ctx matmul.
"""

import numpy as np
import ml_dtypes

import concourse.bacc as bacc
import concourse.mybir as mybir
import concourse.tile as tile
from concourse import bass_utils

BF = ml_dtypes.bfloat16
dt = mybir.dt

NCORES = 8


def _emit(nc, tc, inp, y_d, S, E, HL, DK):
    EC = E // 128          # contraction chunks for the projections
    NT = S // 128          # seq tiles
    DKL = HL * DK          # local head dims (256)
    NP = DKL // 128        # q/k partition tiles (pairs of heads)
    Exp = mybir.ActivationFunctionType.Exp
    Copy = mybir.ActivationFunctionType.Copy
    CS = min(512, S)   # matmul free-dim chunk over seq
    CE = min(512, E)   # matmul free-dim chunk over embed

    persist = tc.alloc_tile_pool(name="persist", bufs=1)
    qT = [persist.tile([128, S], dt.bfloat16, name=f"qT{m}") for m in range(NP)]
    kT = [persist.tile([128, S], dt.bfloat16, name=f"kT{m}") for m in range(NP)]
    cT = [persist.tile([128, S], dt.bfloat16, name=f"cT{m}") for m in range(NP)]
    vA = [persist.tile([128, HL * (DK + 1)], dt.bfloat16, name=f"vA{t}")
          for t in range(NT)]
    ones = persist.tile([1, S], dt.bfloat16, name="ones")
    nc.gpsimd.memset(ones[:], 1.0)
    neg3 = persist.tile([128, 1], dt.float32, name="neg3")
    nc.gpsimd.memset(neg3[:], -3.0)

    w_sb = {}
    b_sb = {}
    for nm in ("wq", "wk", "wv"):
        w_sb[nm] = [persist.tile([128, DKL], dt.bfloat16, name=f"{nm}{c}")
                    for c in range(EC)]
        for c in range(EC):
            nc.sync.dma_start(w_sb[nm][c][:], inp[nm][c])
    for nm in ("bq", "bk", "bv"):
        b_sb[nm] = persist.tile([1, DKL], dt.bfloat16, name=f"{nm}s")
        nc.sync.dma_start(b_sb[nm][:], inp[nm][:])
    wo_sb = [persist.tile([128, E], dt.bfloat16, name=f"wo{p}") for p in range(NP)]
    for p in range(NP):
        nc.sync.dma_start(wo_sb[p][:], inp["wo"][p])

    mpool = tc.alloc_tile_pool(name="mask", bufs=NT)
    mt = []

    # ---- phase 1a: q^T and k^T projections ----------------------------------
    with tc.tile_pool(name="xs", bufs=8) as xs_pool:
        with tc.tile_pool(name="qkps", bufs=2, space="PSUM") as qkps:
            for nm, bias_nm, scale, outtiles in (
                ("xq", "bq", 1.0 / np.sqrt(DK), qT),
                ("xk", "bk", 1.0, kT),
            ):
                xt = []
                for c in range(EC):
                    t_ = xs_pool.tile([128, S], dt.bfloat16, tag="xs",
                                      name=f"{nm}{c}")
                    nc.sync.dma_start(t_[:], inp[nm][c])
                    xt.append(t_)
                for m in range(NP):
                    ps = qkps.tile([128, S], dt.float32, tag="qkps",
                                   name=f"{nm}ps{m}")
                    for n0 in range(0, S, CS):
                        nc.tensor.matmul(
                            ps[:, n0:n0 + CS],
                            b_sb[bias_nm][0:1, 128 * m:128 * (m + 1)],
                            ones[0:1, n0:n0 + CS],
                            start=True, stop=False)
                    for c in range(EC):
                        for n0 in range(0, S, CS):
                            nc.tensor.matmul(
                                ps[:, n0:n0 + CS],
                                w_sb[nm.replace("x", "w")][c][:, 128 * m:128 * (m + 1)],
                                xt[c][:, n0:n0 + CS],
                                start=False, stop=(c == EC - 1))
                    nc.scalar.activation(outtiles[m][:], ps[:], Copy, scale=scale)

        # ---- phase 1b: v (natural layout) with ones column ------------------
        with tc.tile_pool(name="vps", bufs=2, space="PSUM") as vpsp:
            xt = []
            for c in range(EC):
                t_ = xs_pool.tile([128, S], dt.bfloat16, tag="xs", name=f"xv{c}")
                nc.sync.dma_start(t_[:], inp["xv"][c])
                xt.append(t_)
            for t in range(NT):
                m_ = mpool.tile([128, S], dt.bfloat16, tag="mask", name=f"mask{t}")
                nc.sync.dma_start(m_[:], inp["mask"][:, t, :])
                mt.append(m_)
            for t in range(NT):
                vps = vpsp.tile([128, DKL], dt.float32, tag="vps", name=f"vps{t}")
                nc.tensor.matmul(vps[:], ones[0:1, 0:128], b_sb["bv"][:],
                                 start=True, stop=False)
                for c in range(EC):
                    nc.tensor.matmul(vps[:],
                                     xt[c][:, t * 128:(t + 1) * 128],
                                     w_sb["wv"][c][:],
                                     start=False, stop=(c == EC - 1))
                nc.gpsimd.memset(vA[t][:], 1.0)
                nc.vector.tensor_copy(
                    vA[t][:].rearrange("p (h c) -> p h c", h=HL)[:, :, 0:DK],
                    vps[:].rearrange("p (h c) -> p h c", h=HL))

    # ---- phase 2: attention per (head, qi-half) ----------------------------
    HB = min(1024, S)  # qi half-block: st psum = 2 banks, ctx psum = 2 banks
    with tc.tile_pool(name="est", bufs=6) as epool, \
         tc.tile_pool(name="nrm", bufs=2) as npool, \
         tc.tile_pool(name="stps", bufs=2, space="PSUM") as stpool, \
         tc.tile_pool(name="ctxps", bufs=2, space="PSUM") as ctxpool:
        for h in range(HL):
            pair, sub = h // 2, (h % 2) * 64
            for q0 in range(0, S, HB):
                ctx = ctxpool.tile([DK + 1, HB], dt.float32, tag="ctx",
                                   name=f"ctx{h}_{q0}")
                for t in range(NT):
                    st = stpool.tile([128, HB], dt.float32, tag="st",
                                     name=f"st{h}_{t}_{q0}")
                    for n0 in range(0, HB, CS):
                        nc.tensor.matmul(
                            st[:, n0:n0 + CS],
                            kT[pair][sub:sub + DK, t * 128:(t + 1) * 128],
                            qT[pair][sub:sub + DK, q0 + n0:q0 + n0 + CS],
                            start=True, stop=True)
                    e = epool.tile([128, HB], dt.bfloat16, tag="e",
                                   name=f"e{h}_{t}_{q0}")
                    nc.scalar.activation(e[:], st[:], Exp, bias=neg3[:])
                    nc.vector.tensor_mul(e[:], e[:], mt[t][:, q0:q0 + HB])
                    for n0 in range(0, HB, CS):
                        nc.tensor.matmul(
                            ctx[:, n0:n0 + CS],
                            vA[t][:, h * (DK + 1):(h + 1) * (DK + 1)],
                            e[:, n0:n0 + CS],
                            start=(t == 0), stop=(t == NT - 1))
                dn = npool.tile([1, HB], dt.float32, tag="dn", name=f"dn{h}_{q0}")
                nc.vector.tensor_copy(dn[:], ctx[DK:DK + 1, :])
                bd = npool.tile([DK, HB], dt.float32, tag="bd", name=f"bd{h}_{q0}")
                nc.gpsimd.partition_broadcast(bd[:], dn[:])
                bc = npool.tile([DK, HB], dt.float32, tag="bc", name=f"bc{h}_{q0}")
                nc.vector.reciprocal_approx_fast(bc[:], bd[:])
                nc.vector.tensor_mul(cT[pair][sub:sub + DK, q0:q0 + HB],
                                     ctx[0:DK, :], bc[:])

    # ---- phase 3: partial output projection --------------------------------
    with tc.tile_pool(name="yps", bufs=2, space="PSUM") as ypool, \
         tc.tile_pool(name="ysb", bufs=2) as ysbp:
        for s in range(NT):
            yps = ypool.tile([128, E], dt.float32, tag="y", name=f"yps{s}")
            for p in range(NP):
                for n0 in range(0, E, CE):
                    nc.tensor.matmul(yps[:, n0:n0 + CE],
                                     cT[p][:, s * 128:(s + 1) * 128],
                                     wo_sb[p][:, n0:n0 + CE],
                                     start=(p == 0), stop=(p == NP - 1))
            ysb = ysbp.tile([128, E], dt.float32, tag="ysb", name=f"ysb{s}")
            if s % 2 == 0:
                nc.scalar.activation(ysb[:], yps[:], Copy)
            else:
                nc.vector.tensor_copy(ysb[:], yps[:])
            nc.sync.dma_start(y_d[s * 128:(s + 1) * 128, :], ysb[:])

    mpool.release()
    persist.release()


def _build(S, E, HL, DK):
    EC = E // 128
    NT = S // 128
    DKL = HL * DK
    NP = DKL // 128
    nc = bacc.Bacc("TRN2", target_bir_lowering=False, debug=False,
                   num_devices=NCORES)
    inp = {}
    for nm in ("xq", "xk", "xv"):
        inp[nm] = nc.dram_tensor(nm, [EC, 128, S], dt.bfloat16,
                                 kind="ExternalInput").ap()
    for nm in ("wq", "wk", "wv"):
        inp[nm] = nc.dram_tensor(nm, [EC, 128, DKL], dt.bfloat16,
                                 kind="ExternalInput").ap()
    for nm in ("bq", "bk", "bv"):
        inp[nm] = nc.dram_tensor(nm, [1, DKL], dt.bfloat16,
                                 kind="ExternalInput").ap()
    inp["wo"] = nc.dram_tensor("wo", [NP, 128, E], dt.bfloat16,
                               kind="ExternalInput").ap()
    inp["mask"] = nc.dram_tensor("mask", [128, NT, S], dt.bfloat16,
                                 kind="ExternalInput").ap()
    y_d = nc.dram_tensor("y", [S, E], dt.float32, kind="ExternalOutput").ap()

    with tile.TileContext(nc) as tc:
        _emit(nc, tc, inp, y_d, S, E, HL, DK)
    nc.compile()
    return nc


_CACHE = {}
_TRACE = False
_TRACE_CORES = (0,)
_LAST_RESULT = None


def _get_nc(S, E, HL, DK):
    key = (S, E, HL, DK)
    if key not in _CACHE:
        _CACHE[key] = _build(S, E, HL, DK)
    return _CACHE[key]


def run_sharded(query, key, value, mask, Wq, bq, Wk, bk, Wv, bv, Wo, bo):
    """Full-input -> full-output runner (generic shapes)."""
    global _LAST_RESULT
    query, key, value = (np.asarray(a, np.float32) for a in (query, key, value))
    mask = np.asarray(mask)
    Wq, bq, Wk, bk, Wv, bv, Wo, bo = (
        np.asarray(a, np.float32) for a in (Wq, bq, Wk, bk, Wv, bv, Wo, bo))

    B, S, E = query.shape
    HDK = Wq.shape[1]
    DKv = 64
    H = HDK // DKv
    GPB = NCORES // B                 # cores per batch
    HL = H // GPB                     # heads per core
    DKL = HL * DKv
    EC = E // 128
    NT = S // 128

    nc = _get_nc(S, E, HL, DKv)

    # per-batch host prep (shared by the 4 cores of a batch)
    xb = {}
    for b in range(B):
        xb[b] = {
            "xq": np.ascontiguousarray(query[b].T).astype(BF).reshape(EC, 128, S),
            "xk": np.ascontiguousarray(key[b].T).astype(BF).reshape(EC, 128, S),
            "xv": np.ascontiguousarray(value[b].T).astype(BF).reshape(EC, 128, S),
            "mask": np.ascontiguousarray(
                mask[b].reshape(S, NT, 128).transpose(2, 1, 0)).astype(BF),
        }

    in_maps = []
    for c in range(NCORES):
        b, g = c // GPB, c % GPB
        sl = slice(g * DKL, (g + 1) * DKL)
        in_maps.append({
            **xb[b],
            "wq": np.ascontiguousarray(Wq[:, sl]).astype(BF).reshape(EC, 128, DKL),
            "wk": np.ascontiguousarray(Wk[:, sl]).astype(BF).reshape(EC, 128, DKL),
            "wv": np.ascontiguousarray(Wv[:, sl]).astype(BF).reshape(EC, 128, DKL),
            "bq": bq[sl].astype(BF).reshape(1, DKL),
            "bk": bk[sl].astype(BF).reshape(1, DKL),
            "bv": bv[sl].astype(BF).reshape(1, DKL),
            "wo": np.ascontiguousarray(Wo[sl, :]).astype(BF).reshape(
                DKL // 128, 128, E),
        })

    kwargs = {}
    if _TRACE:
        kwargs = dict(trace=True, trace_cores=list(_TRACE_CORES))
    res = bass_utils.run_bass_kernel_spmd(nc, in_maps,
                                          core_ids=list(range(NCORES)), **kwargs)
    _LAST_RESULT = res

    y = np.zeros((B, S, E), np.float32)
    for c in range(NCORES):
        y[c // GPB] += res.results[c]["y"]
    y += bo.astype(np.float32)
    return y


def kernel(**inputs):
    return run_sharded(
        inputs["query"], inputs["key"], inputs["value"], inputs["mask"],
        inputs["Wq"], inputs["bq"], inputs["Wk"], inputs["bk"],
        inputs["Wv"], inputs["bv"], inputs["Wo"], inputs["bo"])


# revision 14
# speedup vs baseline: 1.2338x; 1.0398x over previous
"""Multi-head attention (B=2,S=2048,E=1024,H=16,DK=DV=64) on 8 Trainium2 cores.

Sharding: core c handles batch c//4 and head-group c%4 (4 heads each).
Each core computes q/k/v projections for its heads, masked softmax attention
(transposed-scores layout), and a partial output projection with its rows of
Wo.  The host sums the 4 partial outputs per batch and adds bo.

All matmuls run in bf16 with fp32 PSUM accumulation.  Softmax skips the
running-max (scores are O(1) here; a constant -3 bias in the exp guards the
range), masking is a bf16 multiply after exp, and the softmax denominator is
produced by a ones-column appended to V inside the ctx matmul.
"""

import numpy as np
import ml_dtypes

import concourse.bacc as bacc
import concourse.mybir as mybir
import concourse.tile as tile
from concourse import bass_utils

BF = ml_dtypes.bfloat16
dt = mybir.dt

NCORES = 8


def _emit(nc, tc, inp, y_d, S, E, HL, DK):
    EC = E // 128          # contraction chunks for the projections
    NT = S // 128          # seq tiles
    DKL = HL * DK          # local head dims (256)
    NP = DKL // 128        # q/k partition tiles (pairs of heads)
    Exp = mybir.ActivationFunctionType.Exp
    Copy = mybir.ActivationFunctionType.Copy
    CS = min(512, S)   # matmul free-dim chunk over seq
    CE = min(512, E)   # matmul free-dim chunk over embed

    persist = tc.alloc_tile_pool(name="persist", bufs=1)
    qT = [persist.tile([128, S], dt.bfloat16, name=f"qT{m}") for m in range(NP)]
    kT = [persist.tile([128, S], dt.bfloat16, name=f"kT{m}") for m in range(NP)]
    cT = [persist.tile([128, S], dt.bfloat16, name=f"cT{m}") for m in range(NP)]
    vA = [persist.tile([128, HL * (DK + 1)], dt.bfloat16, name=f"vA{t}")
          for t in range(NT)]
    ones = persist.tile([1, S], dt.bfloat16, name="ones")
    nc.gpsimd.memset(ones[:], 1.0)
    neg3 = persist.tile([128, 1], dt.float32, name="neg3")
    nc.gpsimd.memset(neg3[:], -3.0)

    w_sb = {}
    b_sb = {}
    for nm in ("wq", "wk", "wv"):
        w_sb[nm] = [persist.tile([128, DKL], dt.bfloat16, name=f"{nm}{c}")
                    for c in range(EC)]
        for c in range(EC):
            nc.sync.dma_start(w_sb[nm][c][:], inp[nm][c])
    for nm in ("bq", "bk", "bv"):
        b_sb[nm] = persist.tile([1, DKL], dt.bfloat16, name=f"{nm}s")
        nc.sync.dma_start(b_sb[nm][:], inp[nm][:])
    wo_sb = [persist.tile([128, E], dt.bfloat16, name=f"wo{p}") for p in range(NP)]
    for p in range(NP):
        nc.sync.dma_start(wo_sb[p][:], inp["wo"][p])

    mpool = tc.alloc_tile_pool(name="mask", bufs=NT)
    mt = []

    # ---- phase 1a: q^T and k^T projections ----------------------------------
    with tc.tile_pool(name="xs", bufs=8) as xs_pool:
        with tc.tile_pool(name="qkps", bufs=2, space="PSUM") as qkps:
            for nm, bias_nm, scale, outtiles in (
                ("xq", "bq", 1.0 / np.sqrt(DK), qT),
                ("xk", "bk", 1.0, kT),
            ):
                xt = []
                for c in range(EC):
                    t_ = xs_pool.tile([128, S], dt.bfloat16, tag="xs",
                                      name=f"{nm}{c}")
                    nc.sync.dma_start(t_[:], inp[nm][c])
                    xt.append(t_)
                for m in range(NP):
                    ps = qkps.tile([128, S], dt.float32, tag="qkps",
                                   name=f"{nm}ps{m}")
                    for n0 in range(0, S, CS):
                        nc.tensor.matmul(
                            ps[:, n0:n0 + CS],
                            b_sb[bias_nm][0:1, 128 * m:128 * (m + 1)],
                            ones[0:1, n0:n0 + CS],
                            start=True, stop=False)
                    for c in range(EC):
                        for n0 in range(0, S, CS):
                            nc.tensor.matmul(
                                ps[:, n0:n0 + CS],
                                w_sb[nm.replace("x", "w")][c][:, 128 * m:128 * (m + 1)],
                                xt[c][:, n0:n0 + CS],
                                start=False, stop=(c == EC - 1))
                    nc.scalar.activation(outtiles[m][:], ps[:], Copy, scale=scale)

        # ---- phase 1b: v (natural layout) with ones column ------------------
        with tc.tile_pool(name="vps", bufs=2, space="PSUM") as vpsp:
            xt = []
            for c in range(EC):
                t_ = xs_pool.tile([128, S], dt.bfloat16, tag="xs", name=f"xv{c}")
                nc.sync.dma_start(t_[:], inp["xv"][c])
                xt.append(t_)
            for t in range(NT):
                m_ = mpool.tile([128, S], dt.bfloat16, tag="mask", name=f"mask{t}")
                nc.sync.dma_start(m_[:], inp["mask"][:, t, :])
                mt.append(m_)
            for t in range(NT):
                vps = vpsp.tile([128, DKL], dt.float32, tag="vps", name=f"vps{t}")
                nc.tensor.matmul(vps[:], ones[0:1, 0:128], b_sb["bv"][:],
                                 start=True, stop=False)
                for c in range(EC):
                    nc.tensor.matmul(vps[:],
                                     xt[c][:, t * 128:(t + 1) * 128],
                                     w_sb["wv"][c][:],
                                     start=False, stop=(c == EC - 1))
                nc.gpsimd.memset(vA[t][:], 1.0)
                nc.vector.tensor_copy(
                    vA[t][:].rearrange("p (h c) -> p h c", h=HL)[:, :, 0:DK],
                    vps[:].rearrange("p (h c) -> p h c", h=HL))

    # ---- phase 2: attention, two interleaved head-chains -------------------
    # Chains use different qT/kT pair tiles and disjoint PSUM tags, so the PE
    # always has a second dependency-free stream and stays HAM-warm.
    HB = min(1024, S)  # qi half-block: st psum = 2 banks, ctx psum = 2 banks
    with tc.tile_pool(name="est", bufs=6) as epool, \
         tc.tile_pool(name="nrm", bufs=2) as npool, \
         tc.tile_pool(name="stps", bufs=1, space="PSUM") as stpool, \
         tc.tile_pool(name="ctxps", bufs=1, space="PSUM") as ctxpool:
        NCH = min(2, NP)
        for step in range(HL // NCH):
            chains = [step + 2 * ci for ci in range(NCH)]  # head ids, one per pair
            for q0 in range(0, S, HB):
                ctxs = {}
                for ci, h in enumerate(chains):
                    ctxs[h] = ctxpool.tile([DK + 1, HB], dt.float32,
                                           tag=f"ctx{ci}", name=f"ctx{h}_{q0}")
                for t in range(NT):
                    for ci, h in enumerate(chains):
                        pair, sub = h // 2, (h % 2) * 64
                        st = stpool.tile([128, HB], dt.float32, tag=f"st{ci}",
                                         name=f"st{h}_{t}_{q0}")
                        for n0 in range(0, HB, CS):
                            nc.tensor.matmul(
                                st[:, n0:n0 + CS],
                                kT[pair][sub:sub + DK, t * 128:(t + 1) * 128],
                                qT[pair][sub:sub + DK, q0 + n0:q0 + n0 + CS],
                                start=True, stop=True)
                        e = epool.tile([128, HB], dt.bfloat16, tag="e",
                                       name=f"e{h}_{t}_{q0}")
                        nc.scalar.activation(e[:], st[:], Exp, bias=neg3[:])
                        nc.vector.tensor_mul(e[:], e[:], mt[t][:, q0:q0 + HB])
                        for n0 in range(0, HB, CS):
                            nc.tensor.matmul(
                                ctxs[h][:, n0:n0 + CS],
                                vA[t][:, h * (DK + 1):(h + 1) * (DK + 1)],
                                e[:, n0:n0 + CS],
                                start=(t == 0), stop=(t == NT - 1))
                for ci, h in enumerate(chains):
                    pair, sub = h // 2, (h % 2) * 64
                    ctx = ctxs[h]
                    dn = npool.tile([1, HB], dt.float32, tag="dn",
                                    name=f"dn{h}_{q0}")
                    nc.vector.tensor_copy(dn[:], ctx[DK:DK + 1, :])
                    bd = npool.tile([DK, HB], dt.float32, tag="bd",
                                    name=f"bd{h}_{q0}")
                    nc.gpsimd.partition_broadcast(bd[:], dn[:])
                    bc = npool.tile([DK, HB], dt.float32, tag="bc",
                                    name=f"bc{h}_{q0}")
                    nc.vector.reciprocal_approx_fast(bc[:], bd[:])
                    nc.vector.tensor_mul(cT[pair][sub:sub + DK, q0:q0 + HB],
                                         ctx[0:DK, :], bc[:])

    # ---- phase 3: partial output projection --------------------------------
    with tc.tile_pool(name="yps", bufs=2, space="PSUM") as ypool, \
         tc.tile_pool(name="ysb", bufs=2) as ysbp:
        for s in range(NT):
            yps = ypool.tile([128, E], dt.float32, tag="y", name=f"yps{s}")
            for p in range(NP):
                for n0 in range(0, E, CE):
                    nc.tensor.matmul(yps[:, n0:n0 + CE],
                                     cT[p][:, s * 128:(s + 1) * 128],
                                     wo_sb[p][:, n0:n0 + CE],
                                     start=(p == 0), stop=(p == NP - 1))
            ysb = ysbp.tile([128, E], dt.float32, tag="ysb", name=f"ysb{s}")
            nc.vector.tensor_copy(ysb[:], yps[:])
            nc.sync.dma_start(y_d[s * 128:(s + 1) * 128, :], ysb[:])

    mpool.release()
    persist.release()


def _build(S, E, HL, DK):
    EC = E // 128
    NT = S // 128
    DKL = HL * DK
    NP = DKL // 128
    nc = bacc.Bacc("TRN2", target_bir_lowering=False, debug=False,
                   num_devices=NCORES)
    inp = {}
    for nm in ("xq", "xk", "xv"):
        inp[nm] = nc.dram_tensor(nm, [EC, 128, S], dt.bfloat16,
                                 kind="ExternalInput").ap()
    for nm in ("wq", "wk", "wv"):
        inp[nm] = nc.dram_tensor(nm, [EC, 128, DKL], dt.bfloat16,
                                 kind="ExternalInput").ap()
    for nm in ("bq", "bk", "bv"):
        inp[nm] = nc.dram_tensor(nm, [1, DKL], dt.bfloat16,
                                 kind="ExternalInput").ap()
    inp["wo"] = nc.dram_tensor("wo", [NP, 128, E], dt.bfloat16,
                               kind="ExternalInput").ap()
    inp["mask"] = nc.dram_tensor("mask", [128, NT, S], dt.bfloat16,
                                 kind="ExternalInput").ap()
    y_d = nc.dram_tensor("y", [S, E], dt.float32, kind="ExternalOutput").ap()

    with tile.TileContext(nc) as tc:
        _emit(nc, tc, inp, y_d, S, E, HL, DK)
    nc.compile()
    return nc


_CACHE = {}
_TRACE = False
_TRACE_CORES = (0,)
_LAST_RESULT = None


def _get_nc(S, E, HL, DK):
    key = (S, E, HL, DK)
    if key not in _CACHE:
        _CACHE[key] = _build(S, E, HL, DK)
    return _CACHE[key]


def run_sharded(query, key, value, mask, Wq, bq, Wk, bk, Wv, bv, Wo, bo):
    """Full-input -> full-output runner (generic shapes)."""
    global _LAST_RESULT
    query, key, value = (np.asarray(a, np.float32) for a in (query, key, value))
    mask = np.asarray(mask)
    Wq, bq, Wk, bk, Wv, bv, Wo, bo = (
        np.asarray(a, np.float32) for a in (Wq, bq, Wk, bk, Wv, bv, Wo, bo))

    B, S, E = query.shape
    HDK = Wq.shape[1]
    DKv = 64
    H = HDK // DKv
    GPB = NCORES // B                 # cores per batch
    HL = H // GPB                     # heads per core
    DKL = HL * DKv
    EC = E // 128
    NT = S // 128

    nc = _get_nc(S, E, HL, DKv)

    # per-batch host prep (shared by the 4 cores of a batch)
    xb = {}
    for b in range(B):
        xb[b] = {
            "xq": np.ascontiguousarray(query[b].T).astype(BF).reshape(EC, 128, S),
            "xk": np.ascontiguousarray(key[b].T).astype(BF).reshape(EC, 128, S),
            "xv": np.ascontiguousarray(value[b].T).astype(BF).reshape(EC, 128, S),
            "mask": np.ascontiguousarray(
                mask[b].reshape(S, NT, 128).transpose(2, 1, 0)).astype(BF),
        }

    in_maps = []
    for c in range(NCORES):
        b, g = c // GPB, c % GPB
        sl = slice(g * DKL, (g + 1) * DKL)
        in_maps.append({
            **xb[b],
            "wq": np.ascontiguousarray(Wq[:, sl]).astype(BF).reshape(EC, 128, DKL),
            "wk": np.ascontiguousarray(Wk[:, sl]).astype(BF).reshape(EC, 128, DKL),
            "wv": np.ascontiguousarray(Wv[:, sl]).astype(BF).reshape(EC, 128, DKL),
            "bq": bq[sl].astype(BF).reshape(1, DKL),
            "bk": bk[sl].astype(BF).reshape(1, DKL),
            "bv": bv[sl].astype(BF).reshape(1, DKL),
            "wo": np.ascontiguousarray(Wo[sl, :]).astype(BF).reshape(
                DKL // 128, 128, E),
        })

    kwargs = {}
    if _TRACE:
        kwargs = dict(trace=True, trace_cores=list(_TRACE_CORES))
    res = bass_utils.run_bass_kernel_spmd(nc, in_maps,
                                          core_ids=list(range(NCORES)), **kwargs)
    _LAST_RESULT = res

    y = np.zeros((B, S, E), np.float32)
    for c in range(NCORES):
        y[c // GPB] += res.results[c]["y"]
    y += bo.astype(np.float32)
    return y


def kernel(**inputs):
    return run_sharded(
        inputs["query"], inputs["key"], inputs["value"], inputs["mask"],
        inputs["Wq"], inputs["bq"], inputs["Wk"], inputs["bk"],
        inputs["Wv"], inputs["bv"], inputs["Wo"], inputs["bo"])
